# revision 1
# baseline (speedup 1.0000x reference)
"""VQ codebook (nn_Codebook) Trainium2 kernel — self-contained.

kernel(z, emb) -> (zq_out, idx, loss), matching the jax reference:
    zp = z.transpose(0,2,3,1); zf = zp.reshape(-1, C)
    d = |z|^2 + |e|^2 - 2 z e^T ; idx = argmin(d, 1)
    zq = emb[idx]; loss = mean((zq-zp)^2) - 0.25*mean((zp-zq)^2)
    zq_out = (zp + stop_grad(zq - zp)).transpose(0,3,1,2)

Sharding: data-parallel over 8 NeuronCores, 4 batches (16384 tokens) per
core; the [1024, 256] codebook is replicated.  Each core computes
s' = (C2 - |z|^2) - |e|^2  (argmax(s') == argmin(d), bit-exact vs the
reference's fp32 rounding — validated offline), via fp32 PE matmuls with
z kept in its native [C, H*W] layout, then DVE max/max_index (first-index
tie-break == jnp.argmin), an SBUF-source fp16 gather of the codebook for
the straight-through output, and per-lane loss partial sums finished in
fp64 on the host.
"""

import numpy as np
from contextlib import ExitStack

import concourse.bacc as bacc
import concourse.bass as bass
import concourse.mybir as mybir
import concourse.tile as tile
from concourse.bass_utils import run_bass_kernel_spmd

F32 = mybir.dt.float32
F16 = mybir.dt.float16
I16 = mybir.dt.int16
U32 = mybir.dt.uint32

B = 32           # batches
D = 256          # latent dim
K = 1024         # codes
HW = 4096        # tokens per batch (64*64)
N_CORES = 8
B_CORE = B // N_CORES
TOK_TILE = 128
GROUP = 512
BETA = 0.25

_CACHED = {}


def _build_kernel():
    n_tok = B_CORE * HW
    n_tiles = n_tok // TOK_TILE
    n_groups = n_tok // GROUP
    tpg = GROUP // TOK_TILE  # tiles per group

    nc = bacc.Bacc("TRN2", target_bir_lowering=False, debug=False,
                   num_devices=N_CORES)

    z_d = nc.dram_tensor("z", [B_CORE, 2, 128, HW], F32, kind="ExternalInput")
    znn_d = nc.dram_tensor("znn", [128, n_tiles], F32, kind="ExternalInput")
    e2t_d = nc.dram_tensor("e2t", [2, 128, K], F32, kind="ExternalInput")
    eb_d = nc.dram_tensor("eb", [128, K], F32, kind="ExternalInput")
    et16_d = nc.dram_tensor("et16", [128, 8 * 256], F16, kind="ExternalInput")

    zq_d = nc.dram_tensor("zq", [B_CORE, 2, 128, HW], F32, kind="ExternalOutput")
    idx_d = nc.dram_tensor("idx", [n_tok], U32, kind="ExternalOutput")
    lp_d = nc.dram_tensor("lp", [128, 1], F32, kind="ExternalOutput")

    scrA_d = nc.dram_tensor("scrA", [n_groups, GROUP], I16)
    scrB_d = nc.dram_tensor("scrB", [n_groups, 8, GROUP], I16)

    with tile.TileContext(nc) as tc, ExitStack() as ctx:
        cpool = ctx.enter_context(tc.tile_pool(name="const", bufs=1))
        zpool = ctx.enter_context(tc.tile_pool(name="zin", bufs=3))
        spool = ctx.enter_context(tc.tile_pool(name="scores", bufs=3))
        qpool = ctx.enter_context(tc.tile_pool(name="zq", bufs=2))
        ipool = ctx.enter_context(tc.tile_pool(name="idxs", bufs=2))
        pspool = ctx.enter_context(tc.tile_pool(name="ps", bufs=3, space="PSUM"))

        eb_sb = cpool.tile([128, K], F32)
        nc.sync.dma_start(eb_sb[:], eb_d.ap())
        znn_sb = cpool.tile([128, n_tiles], F32)
        nc.sync.dma_start(znn_sb[:], znn_d.ap())
        e2t_sb = cpool.tile([128, 2, K], F32)
        nc.sync.dma_start(e2t_sb[:], e2t_d.ap().rearrange("c d k -> d c k"))
        et16_sb = cpool.tile([128, 8 * 256], F16)
        nc.sync.dma_start(et16_sb[:], et16_d.ap())
        macc = cpool.tile([128, n_tiles], F32)

        for g in range(n_groups):
            b = (g * GROUP) // HW
            j0 = (g * GROUP) % HW
            zt = zpool.tile([128, 2, GROUP], F32, tag="zt")
            nc.sync.dma_start(zt[:], z_d.ap()[b, :, :, j0:j0 + GROUP]
                              .rearrange("c d t -> d c t"))

            idxg32 = ipool.tile([128, tpg, 8], U32, tag="idxg32")
            idxg16 = ipool.tile([128, 64], I16, tag="idxg16")

            for ti in range(tpg):
                t = g * tpg + ti
                tok0 = ti * TOK_TILE
                ps = pspool.tile([128, 2, 512], F32, tag="ps")
                for h in range(2):
                    nc.tensor.matmul(ps[:, h, :],
                                     zt[:, 0, tok0:tok0 + TOK_TILE],
                                     e2t_sb[:, 0, 512 * h:512 * (h + 1)],
                                     start=True, stop=False)
                    nc.tensor.matmul(ps[:, h, :],
                                     zt[:, 1, tok0:tok0 + TOK_TILE],
                                     e2t_sb[:, 1, 512 * h:512 * (h + 1)],
                                     start=False, stop=True)
                c2u = spool.tile([128, K], F32, tag="c2u")
                nc.scalar.activation(c2u[:], ps[:].rearrange("p a b -> p (a b)"),
                                     mybir.ActivationFunctionType.Identity,
                                     bias=znn_sb[:, t:t + 1], scale=1.0)
                ssb = spool.tile([128, K], F32, tag="ssb")
                nc.gpsimd.tensor_sub(ssb[:], c2u[:], eb_sb[:])
                nc.vector.tensor_reduce(macc[:, t:t + 1], ssb[:],
                                        axis=mybir.AxisListType.X,
                                        op=mybir.AluOpType.max)
                nc.vector.max_index(idxg32[:, ti, :],
                                    macc[:, t:t + 1].to_broadcast((128, 8)),
                                    ssb[:])
                nc.vector.tensor_copy(idxg16[:, ti:ti + 1], idxg32[:, ti, 0:1])

            ia = idxg32[:, :, 0:1]
            ctx_nc = nc.allow_non_contiguous_dma(reason="idx scatter, small")
            ctx_nc.__enter__()
            nc.sync.dma_start(
                bass.AP(idx_d, g * GROUP, [[1, 128], [128, tpg], [1, 1]]),
                bass.AP(ia.tensor, ia.offset,
                        [[ia.ap[0][0], 128], [8, tpg], [1, 1]]))
            sa = idxg16[:, 0:tpg]
            nc.scalar.dma_start(
                bass.AP(scrA_d, g * GROUP, [[4, 128], [1, 4]]),
                bass.AP(sa.tensor, sa.offset, [[sa.ap[0][0], 128], [1, 4]]))
            # wrap: scrB[g, 0, 32*(i%16) + i//16] = idx of token i = 128*ti+p
            for ti in range(tpg):
                eng = (nc.scalar, nc.sync, nc.scalar, nc.sync)[ti]
                eng.dma_start(
                    bass.AP(scrB_d, g * 8 * GROUP + 8 * ti,
                            [[1, 8], [32, 16], [1, 1]]),
                    bass.AP(scrA_d, g * GROUP + ti,
                            [[64, 8], [4, 16], [1, 1]]))
            nc.scalar.dma_start(
                bass.AP(scrB_d, g * 8 * GROUP + GROUP, [[1, GROUP], [1, 1]]),
                bass.AP(scrB_d, g * 8 * GROUP, [[1, GROUP], [1, 1]]))
            nc.sync.dma_start(
                bass.AP(scrB_d, g * 8 * GROUP + 2 * GROUP, [[1, 2 * GROUP], [1, 1]]),
                bass.AP(scrB_d, g * 8 * GROUP, [[1, 2 * GROUP], [1, 1]]))
            nc.scalar.dma_start(
                bass.AP(scrB_d, g * 8 * GROUP + 4 * GROUP, [[1, 4 * GROUP], [1, 1]]),
                bass.AP(scrB_d, g * 8 * GROUP, [[1, 4 * GROUP], [1, 1]]))
            ctx_nc.__exit__(None, None, None)
            idxw = ipool.tile([128, 32], I16, tag="idxw")
            nc.scalar.dma_start(idxw[:],
                                bass.AP(scrB_d, g * 8 * GROUP, [[32, 128], [1, 32]]))
            zq16 = qpool.tile([128, 2, GROUP], F16, tag="zq16")
            nc.gpsimd.dma_gather(zq16[:], et16_sb[:], idxw[:],
                                 num_idxs=GROUP, num_idxs_reg=GROUP,
                                 elem_size=256, transpose=True,
                                 sbuf_tokens_per_rank=128,
                                 sbuf_free_dim_per_rank=512)
            u = qpool.tile([128, 2, GROUP], F32, tag="u")
            nc.gpsimd.tensor_sub(u[:], zq16[:], zt[:])
            zo = qpool.tile([128, 2, GROUP], F32, tag="zo")
            nc.gpsimd.tensor_add(zo[:], zt[:], u[:])
            nc.sync.dma_start(zq_d.ap()[b, :, :, j0:j0 + GROUP]
                              .rearrange("c d t -> d c t"), zo[:])

        lp = cpool.tile([128, 1], F32)
        nc.vector.tensor_reduce(lp[:], macc[:], axis=mybir.AxisListType.X,
                                op=mybir.AluOpType.add)
        nc.sync.dma_start(lp_d.ap(), lp[:])

    nc.compile()
    return nc


def _prepare_inputs(z_full, emb):
    e64 = emb.astype(np.float64)
    ebias = (e64 * e64).sum(1).astype(np.float32)
    eb128 = np.broadcast_to(ebias, (128, K)).copy()
    e2t = np.ascontiguousarray((2.0 * emb.T).astype(np.float32).reshape(2, 128, K))
    emb16 = emb.astype(np.float16)
    et16 = np.zeros((128, 8 * 256), np.float16)
    for r in range(8):
        et16[:, r * 256:(r + 1) * 256] = emb16[r * 128:(r + 1) * 128, :]
    znorm = (z_full.astype(np.float64) ** 2).sum(1).astype(np.float32)
    znorm = znorm.reshape(B, HW)
    in_maps = []
    for c in range(N_CORES):
        zb = z_full[c * B_CORE:(c + 1) * B_CORE]
        zn = znorm[c * B_CORE:(c + 1) * B_CORE].reshape(-1)
        n_tiles = zn.size // TOK_TILE
        znn = -zn.reshape(n_tiles, TOK_TILE).T.copy()
        in_maps.append({
            "z": np.ascontiguousarray(zb.reshape(B_CORE, 2, 128, HW)),
            "znn": znn, "e2t": e2t, "eb": eb128, "et16": et16,
        })
    return in_maps


def kernel(z, emb):
    z = np.ascontiguousarray(np.asarray(z, dtype=np.float32))
    emb = np.ascontiguousarray(np.asarray(emb, dtype=np.float32))
    assert z.shape == (B, D, 64, 64) and emb.shape == (K, D)

    if "nc" not in _CACHED:
        _CACHED["nc"] = _build_kernel()
    nc = _CACHED["nc"]
    in_maps = _prepare_inputs(z.reshape(B, D, HW), emb)

    last_err = None
    for _attempt in range(3):
        try:
            res = run_bass_kernel_spmd(nc, in_maps, core_ids=list(range(N_CORES)))
            break
        except Exception as e:  # transient device errors: retry
            last_err = e
    else:
        raise last_err

    results = res.results
    zq = np.concatenate(
        [r["zq"].reshape(B_CORE, D, 64, 64) for r in results], 0)
    idx = np.concatenate([r["idx"] for r in results]).astype(np.int32)
    total = -sum(float(r["lp"].astype(np.float64).sum()) for r in results)
    mean = total / (B * HW * D)
    loss = np.float32(mean - BETA * mean)
    return zq, idx, loss
